# revision 13
# baseline (speedup 1.0000x reference)
"""DeepSeek MLA attention (B=1, T=2048, D=2048, H=16) on 8 trn2 NeuronCores.

Two-launch sequence-parallel sharding (faster than plain head-TP: it avoids
replicating the q_a/kv_a input projections on every core).

Launch 1 (token-sharded): each core computes q/k/v for ALL 16 heads for its
256-token slice (no replicated projections). Launch 2 (query-sharded): each
core runs attention for its 256 queries over ALL heads and ALL 2048 keys,
then o_proj with the full wo, producing complete output rows (no all-reduce).
The host reshuffles k/v between launches (concat along tokens).
"""

import sys

if "/opt/trn_rl_repo" not in sys.path:
    sys.path.insert(0, "/opt/trn_rl_repo")

from contextlib import ExitStack

import ml_dtypes
import numpy as np

import concourse.bass as bass
import concourse.tile as tile
from concourse import bacc, mybir
from concourse.bass import ts
from concourse.bass_utils import run_bass_kernel_spmd
from concourse.masks import make_identity

BF16 = ml_dtypes.bfloat16
F32 = mybir.dt.float32
BF = mybir.dt.bfloat16

T, D = 2048, 2048
H, DK = 16, 128
ROPE, NOPE, VD = 64, 64, 128
HALF = ROPE // 2
KVR, QR = 512, 768
KVW = KVR + 128
SCALE = DK ** -0.5
EPS = float(np.finfo(np.float32).eps)
N_CORES = 8
TSL = T // N_CORES        # 256 tokens per core
NTT = T // 128            # 16 token tiles total
P = 128

QRT = QR // P   # 6
CRT = KVR // P  # 4
DT = D // P     # 16


def _emit_l1(nc, tc, ctx, aps):
    (xTs, wqa, wkva, wqb, wkvbn, wkvbv, cosA, sinA2, swapM,
     qsT_o, ksT_o, v_o) = aps
    TCH = TSL            # single 256-wide chunk
    TPC = TCH // P       # 2

    consts = ctx.enter_context(tc.tile_pool(name="consts", bufs=1))
    persist = ctx.enter_context(tc.tile_pool(name="persist", bufs=1))
    wpool = ctx.enter_context(tc.tile_pool(name="w1", bufs=1))
    rawpool = ctx.enter_context(tc.tile_pool(name="raw", bufs=1))
    sqpool = ctx.enter_context(tc.tile_pool(name="sq", bufs=3))
    scpool = ctx.enter_context(tc.tile_pool(name="scales", bufs=1))
    tmpool = ctx.enter_context(tc.tile_pool(name="ropetmp", bufs=2))
    outp = ctx.enter_context(tc.tile_pool(name="outs", bufs=4))
    ps_proj = ctx.enter_context(tc.tile_pool(name="ps_proj", bufs=3, space="PSUM"))
    ps_sum = ctx.enter_context(tc.tile_pool(name="ps_sum", bufs=1, space="PSUM"))
    ps_small = ctx.enter_context(tc.tile_pool(name="ps_small", bufs=2, space="PSUM"))

    ones128 = consts.tile([P, P], BF)
    nc.gpsimd.memset(ones128, 1.0)
    ident = consts.tile([P, P], F32)
    make_identity(nc, ident)
    eps_ap = consts.tile([P, 1], F32)
    nc.vector.memset(eps_ap, EPS)

    s_cT = persist.tile([P, TPC], F32)

    # interleave x tiles with projection weights so matmuls start early
    xc = persist.tile([P, DT, TCH], BF)
    wqa_sb = wpool.tile([P, DT, QR], BF)
    wkva_sb = wpool.tile([P, DT, KVW], BF)
    for dt in range(DT):
        nc.sync.dma_start(out=xc[:, dt, :], in_=xTs[ts(dt, P), :])
        nc.sync.dma_start(out=wqa_sb[:, dt, :], in_=wqa[ts(dt, P), :])
        nc.sync.dma_start(out=wkva_sb[:, dt, :], in_=wkva[ts(dt, P), :])
    wqb_sb = wpool.tile([P, QRT, H, DK], BF)
    nc.sync.dma_start(out=wqb_sb, in_=wqb.rearrange("(r p) h d -> p r h d", p=P))
    wkvbn_sb = wpool.tile([P, CRT, H, NOPE], BF)
    nc.sync.dma_start(out=wkvbn_sb, in_=wkvbn.rearrange("(r p) h d -> p r h d", p=P))
    wkvbv_sb = wpool.tile([P, CRT, H, VD], BF)
    nc.sync.dma_start(out=wkvbv_sb, in_=wkvbv.rearrange("(r p) h d -> p r h d", p=P))
    cosA_sb = wpool.tile([P, TCH], BF)
    nc.sync.dma_start(out=cosA_sb, in_=cosA)
    sinA2_sb = wpool.tile([P, TCH], BF)
    nc.sync.dma_start(out=sinA2_sb, in_=sinA2)
    swapM_sb = wpool.tile([P, P], BF)
    nc.sync.dma_start(out=swapM_sb, in_=swapM)

    qa_r = rawpool.tile([P, QRT, TCH], BF, tag="qa_raw")
    c_r = rawpool.tile([P, CRT, TCH], BF, tag="c_raw")
    kpe_r = rawpool.tile([P, TCH], BF, tag="kpe_raw")
    sq_q_ps = ps_sum.tile([P, TCH], F32, tag="sq_q")
    sq_c_ps = ps_sum.tile([P, TCH], F32, tag="sq_c")

    scale_q = scpool.tile([P, TCH], F32, tag="scale_q")
    scale_c = scpool.tile([P, TCH], F32, tag="scale_c")

    pending = None

    def flush_pending():
        nonlocal pending
        if pending is not None:
            tgt, sqt, st, sp = pending
            nc.tensor.matmul(tgt, ones128, sqt, start=st, stop=sp)
            pending = None

    for r in range(QRT + CRT + 1):
        ps = ps_proj.tile([P, TCH], F32, tag="proj")
        if r < QRT:
            w, col = wqa_sb, ts(r, P)
        elif r < QRT + CRT:
            w, col = wkva_sb, ts(r - QRT, P)
        else:
            w, col = wkva_sb, ts(CRT, P)
        for dt in range(DT):
            nc.tensor.matmul(ps, w[:, dt, col], xc[:, dt, :],
                             start=(dt == 0), stop=(dt == DT - 1))
        flush_pending()
        if r < QRT:
            nc.scalar.copy(out=qa_r[:, r, :], in_=ps)
            sq = sqpool.tile([P, TCH], BF, tag="sq")
            nc.scalar.activation(out=sq, in_=ps,
                                 func=mybir.ActivationFunctionType.Square)
            pending = (sq_q_ps, sq, r == 0, r == QRT - 1)
        elif r < QRT + CRT:
            rc = r - QRT
            nc.scalar.copy(out=c_r[:, rc, :], in_=ps)
            sq = sqpool.tile([P, TCH], BF, tag="sq")
            nc.scalar.activation(out=sq, in_=ps,
                                 func=mybir.ActivationFunctionType.Square)
            pending = (sq_c_ps, sq, rc == 0, rc == CRT - 1)
        else:
            nc.scalar.copy(out=kpe_r, in_=ps)
        if r == QRT:
            tmp_q = scpool.tile([P, TCH], F32, tag="scale_tmp")
            nc.scalar.activation(out=tmp_q, in_=sq_q_ps,
                                 func=mybir.ActivationFunctionType.Sqrt,
                                 scale=1.0 / QR, bias=eps_ap)
            nc.vector.reciprocal(out=scale_q, in_=tmp_q)
    flush_pending()
    tmp_c = scpool.tile([P, TCH], F32, tag="scale_tmp")
    nc.scalar.activation(out=tmp_c, in_=sq_c_ps,
                         func=mybir.ActivationFunctionType.Sqrt,
                         scale=1.0 / KVR, bias=eps_ap)
    nc.vector.reciprocal(out=scale_c, in_=tmp_c)

    cos_s = scpool.tile([P, TCH], BF, tag="cos_s")
    sin_s = scpool.tile([P, TCH], BF, tag="sin_s")
    nc.vector.tensor_mul(cos_s[64:128, :], cosA_sb[64:128, :], scale_q[64:128, :])
    nc.vector.tensor_mul(sin_s[64:128, :], sinA2_sb[64:128, :], scale_q[64:128, :])

    # --- all qsT matmul groups (PE-dense), evictions on ACT/DVE ---
    qs_all = persist.tile([P, H, TCH], BF)
    qpe_all = persist.tile([P, H, TCH], BF)
    for h in range(H):
        ps = ps_proj.tile([P, TCH], F32, tag="proj")
        for r in range(QRT):
            nc.tensor.matmul(ps, wqb_sb[:, r, h, :], qa_r[:, r, :],
                             start=(r == 0), stop=(r == QRT - 1))
        nc.vector.tensor_mul(qs_all[0:64, h, :], ps[0:64, :], scale_q[0:64, :])
        nc.scalar.copy(out=qpe_all[:, h, :], in_=ps)

    # k_nope groups
    ks_l = []
    for h in range(H):
        ks_sb = outp.tile([P, TCH], BF, tag=f"ks{h % 4}")
        kn_ps = ps_small.tile([64, TCH], F32, tag="small")
        for r in range(CRT):
            nc.tensor.matmul(kn_ps, wkvbn_sb[:, r, h, :], c_r[:, r, :],
                             start=(r == 0), stop=(r == CRT - 1))
        nc.vector.tensor_mul(ks_sb[0:64, :], kn_ps, scale_c[0:64, :])
        ks_l.append(ks_sb)

    # scale_c column layout for v scaling
    for j in range(TPC):
        tr_ps = ps_small.tile([P, P], F32, tag="small")
        nc.tensor.transpose(tr_ps, scale_c[:, ts(j, P)], ident)
        nc.vector.tensor_copy(out=s_cT[:, j:j + 1], in_=tr_ps[:, 0:1])

    # shared roped k_pe
    ksw_ps = ps_proj.tile([P, TCH], F32, tag="proj")
    nc.tensor.matmul(ksw_ps, swapM_sb, kpe_r, start=True, stop=True)
    kpe_rope = persist.tile([P, TCH], BF)
    km1 = tmpool.tile([P, TCH], BF, tag="rope_m1")
    km2 = tmpool.tile([P, TCH], BF, tag="rope_m2")
    nc.vector.tensor_mul(km1[64:128, :], kpe_r[64:128, :], cosA_sb[64:128, :])
    nc.vector.tensor_mul(km2[64:128, :], ksw_ps[64:128, :], sinA2_sb[64:128, :])
    nc.vector.tensor_add(kpe_rope[64:128, :], km1[64:128, :], km2[64:128, :])
    for h in range(H):
        nc.vector.tensor_copy(out=ks_l[h][64:128, :], in_=kpe_rope[64:128, :])
        nc.sync.dma_start(out=ksT_o[h], in_=ks_l[h])

    # q rope swaps last (inputs evicted long before)
    for h in range(H):
        sw_ps = ps_proj.tile([P, TCH], F32, tag="proj")
        nc.tensor.matmul(sw_ps, swapM_sb, qpe_all[:, h, :], start=True, stop=True)
        m1 = tmpool.tile([P, TCH], BF, tag="rope_m1")
        m2 = tmpool.tile([P, TCH], BF, tag="rope_m2")
        nc.vector.tensor_mul(m1[64:128, :], qpe_all[64:128, h, :], cos_s[64:128, :])
        nc.vector.tensor_mul(m2[64:128, :], sw_ps[64:128, :], sin_s[64:128, :])
        nc.vector.tensor_add(qs_all[64:128, h, :], m1[64:128, :], m2[64:128, :])
        nc.sync.dma_start(out=qsT_o[h], in_=qs_all[:, h, :])


    # v groups (scale folded into the ACT eviction)
    for h in range(H):
        for j in range(TPC):
            v_ps = ps_small.tile([P, VD], F32, tag="small")
            for r in range(CRT):
                nc.tensor.matmul(v_ps, c_r[:, r, ts(j, P)], wkvbv_sb[:, r, h, :],
                                 start=(r == 0), stop=(r == CRT - 1))
            v_sb = outp.tile([P, VD], BF, tag="v")
            nc.scalar.activation(out=v_sb, in_=v_ps,
                                 func=mybir.ActivationFunctionType.Copy,
                                 scale=s_cT[:, j:j + 1])
            nc.sync.dma_start(out=v_o[h, j], in_=v_sb)

def _emit_l2(nc, tc, ctx, aps):
    (qsT_i, ksT_i, v_i, wo_f, o_out) = aps
    KPACK = 4           # k-tiles packed per exp op (4*256 = 1024 cols, 2 banks)

    consts = ctx.enter_context(tc.tile_pool(name="consts", bufs=1))
    persist = ctx.enter_context(tc.tile_pool(name="persist", bufs=1))
    wpool = ctx.enter_context(tc.tile_pool(name="w2", bufs=1))

    ones128 = consts.tile([P, P], BF)
    nc.gpsimd.memset(ones128, 1.0)

    qs_all = persist.tile([P, H, TSL], BF)
    nc.sync.dma_start(out=qs_all, in_=qsT_i.rearrange("h p t -> p h t"))
    outT = persist.tile([P, H, TSL], BF)

    wo_sb = wpool.tile([P, H, D], BF)

    with tc.tile_pool(name="kv", bufs=3) as kvpool, \
         tc.tile_pool(name="probs", bufs=4) as ppool, \
         tc.tile_pool(name="recip", bufs=3) as recpool, \
         tc.tile_pool(name="ps_att", bufs=2, space="PSUM") as ps_att, \
         tc.tile_pool(name="ps_acc", bufs=2, space="PSUM") as ps_acc:
        pend = None    # (pT, base kt, av_ps, dn_ps, v_tile)

        def flush_av(nc, KPACK):
            nonlocal pend
            if pend is None:
                return
            pT, kb, av_p, dn_p, v_t = pend
            for kl in range(KPACK):
                kt = kb + kl
                nc.tensor.matmul(av_p, v_t[:, kt, :], pT[:, kl, :],
                                 start=(kt == 0), stop=(kt == NTT - 1))
                nc.tensor.matmul(dn_p, ones128, pT[:, kl, :],
                                 start=(kt == 0), stop=(kt == NTT - 1))
            pend = None

        finals = []
        for h in range(H):
            ks_h = kvpool.tile([P, T], BF, tag="ks")
            nc.sync.dma_start(out=ks_h, in_=ksT_i[h])
            v_h = kvpool.tile([P, NTT, VD], BF, tag="v")
            nc.sync.dma_start(out=v_h, in_=v_i[h].rearrange("kt p d -> p kt d"))
            if h == 1:
                # big o_proj weight load rides behind the first head's k/v
                nc.sync.dma_start(out=wo_sb, in_=wo_f.rearrange("h p d -> p h d"))

            av_ps = ps_acc.tile([P, TSL], F32, tag="av")
            dn_ps = ps_acc.tile([P, TSL], F32, tag="dn")
            for kg in range(NTT // KPACK):
                sc_ps = ps_att.tile([P, KPACK, TSL], F32, tag="scores")
                for kl in range(KPACK):
                    kt = kg * KPACK + kl
                    nc.tensor.matmul(sc_ps[:, kl, :], ks_h[:, ts(kt, P)],
                                     qs_all[:, h, :], start=True, stop=True)
                pT = ppool.tile([P, KPACK, TSL], BF, tag="pT")
                nc.scalar.activation(out=pT, in_=sc_ps,
                                     func=mybir.ActivationFunctionType.Exp,
                                     scale=SCALE)
                flush_av(nc, KPACK)
                pend = (pT, kg * KPACK, av_ps, dn_ps, v_h)
                while finals:
                    f_av, f_dn, f_h = finals.pop()
                    rec = recpool.tile([P, TSL], F32, tag="rec")
                    nc.vector.reciprocal(out=rec, in_=f_dn)
                    nc.vector.tensor_mul(outT[:, f_h, :], f_av, rec)
            finals.append((av_ps, dn_ps, h))
        flush_av(nc, KPACK)
        for f_av, f_dn, f_h in finals:
            rec = recpool.tile([P, TSL], F32, tag="rec")
            nc.vector.reciprocal(out=rec, in_=f_dn)
            nc.vector.tensor_mul(outT[:, f_h, :], f_av, rec)

    with tc.tile_pool(name="osb", bufs=3) as opool, \
         tc.tile_pool(name="ps_o", bufs=2, space="PSUM") as ps_o:
        for qt in range(TSL // P):
            o_ps = ps_o.tile([P, D // 512, 512], F32, tag="o")
            for h in range(H):
                for dc in range(D // 512):
                    nc.tensor.matmul(o_ps[:, dc, :], outT[:, h, ts(qt, P)],
                                     wo_sb[:, h, ts(dc, 512)],
                                     start=(h == 0), stop=(h == H - 1))
            o_sb = opool.tile([P, D], F32, tag="osb")
            nc.vector.tensor_copy(out=o_sb, in_=o_ps)
            nc.sync.dma_start(out=o_out[ts(qt, P), :], in_=o_sb)


_CACHE = {}


def _build_l1():
    if "l1" in _CACHE:
        return _CACHE["l1"]
    nc = bacc.Bacc("TRN2", target_bir_lowering=False, debug=False,
                   num_devices=N_CORES)
    xTs = nc.dram_tensor("xTs", [D, TSL], BF, kind="ExternalInput").ap()
    wqa = nc.dram_tensor("wqa", [D, QR], BF, kind="ExternalInput").ap()
    wkva = nc.dram_tensor("wkva", [D, KVW], BF, kind="ExternalInput").ap()
    wqb = nc.dram_tensor("wqb", [QR, H, DK], BF, kind="ExternalInput").ap()
    wkvbn = nc.dram_tensor("wkvbn", [KVR, H, NOPE], BF, kind="ExternalInput").ap()
    wkvbv = nc.dram_tensor("wkvbv", [KVR, H, VD], BF, kind="ExternalInput").ap()
    cosA = nc.dram_tensor("cosA", [P, TSL], BF, kind="ExternalInput").ap()
    sinA2 = nc.dram_tensor("sinA2", [P, TSL], BF, kind="ExternalInput").ap()
    swapM = nc.dram_tensor("swapM", [P, P], BF, kind="ExternalInput").ap()
    qsT_o = nc.dram_tensor("qsT_o", [H, P, TSL], BF, kind="ExternalOutput").ap()
    ksT_o = nc.dram_tensor("ksT_o", [H, P, TSL], BF, kind="ExternalOutput").ap()
    v_o = nc.dram_tensor("v_o", [H, TSL // P, P, VD], BF, kind="ExternalOutput").ap()
    aps = (xTs, wqa, wkva, wqb, wkvbn, wkvbv, cosA, sinA2, swapM,
           qsT_o, ksT_o, v_o)
    with tile.TileContext(nc) as tc, ExitStack() as ctx:
        _emit_l1(nc, tc, ctx, aps)
    nc.compile()
    _CACHE["l1"] = nc
    return nc


def _build_l2():
    if "l2" in _CACHE:
        return _CACHE["l2"]
    nc = bacc.Bacc("TRN2", target_bir_lowering=False, debug=False,
                   num_devices=N_CORES)
    qsT_i = nc.dram_tensor("qsT_i", [H, P, TSL], BF, kind="ExternalInput").ap()
    ksT_i = nc.dram_tensor("ksT_i", [H, P, T], BF, kind="ExternalInput").ap()
    v_i = nc.dram_tensor("v_i", [H, NTT, P, VD], BF, kind="ExternalInput").ap()
    wo_f = nc.dram_tensor("wo_f", [H, VD, D], BF, kind="ExternalInput").ap()
    o_out = nc.dram_tensor("o", [TSL, D], F32, kind="ExternalOutput").ap()
    aps = (qsT_i, ksT_i, v_i, wo_f, o_out)
    with tile.TileContext(nc) as tc, ExitStack() as ctx:
        _emit_l2(nc, tc, ctx, aps)
    nc.compile()
    _CACHE["l2"] = nc
    return nc


def _host_prep(x, wq_a, q_a_norm_w, wq_b, wkv_a, kv_a_norm_w, wkv_b, wo):
    x2 = np.asarray(x, np.float32).reshape(T, D)
    xT_np = np.ascontiguousarray(x2.T).astype(BF16)
    wqa_np = np.asarray(wq_a, np.float32).astype(BF16)
    wkva_f = np.asarray(wkv_a, np.float32)
    wkva_np = np.zeros((D, KVW), BF16)
    wkva_np[:, :KVR] = wkva_f[:, :KVR].astype(BF16)
    wkva_np[:, KVR + 64:] = wkva_f[:, KVR:].astype(BF16)
    wqb_f = (np.asarray(q_a_norm_w, np.float32)[:, None]
             * np.asarray(wq_b, np.float32)).reshape(QR, H, DK).astype(BF16)
    wkvb_f = (np.asarray(kv_a_norm_w, np.float32)[:, None]
              * np.asarray(wkv_b, np.float32)).reshape(KVR, H, NOPE + VD)
    wkvbn_np = np.ascontiguousarray(wkvb_f[:, :, :NOPE]).astype(BF16)
    wkvbv_np = np.ascontiguousarray(wkvb_f[:, :, NOPE:]).astype(BF16)
    wo_np = np.asarray(wo, np.float32).reshape(H, VD, D).astype(BF16)

    inv_freq = 1.0 / (10000.0 ** (np.arange(0, ROPE, 2, dtype=np.float32) / ROPE))
    tpos = np.arange(T, dtype=np.float32)
    freqs = np.outer(inv_freq, tpos)
    cos = np.cos(freqs).astype(np.float32)
    sin = np.sin(freqs).astype(np.float32)
    cosA_np = np.concatenate([cos, cos, cos, cos], 0).astype(BF16)
    sin2 = np.concatenate([-sin, sin], 0)
    sinA2_np = np.concatenate([sin2, sin2], 0).astype(BF16)
    swap_np = np.zeros((P, P), np.float32)
    for i in range(HALF):
        swap_np[64 + HALF + i, 64 + i] = 1.0
        swap_np[64 + i, 64 + HALF + i] = 1.0
    swapM_np = swap_np.astype(BF16)
    return (xT_np, wqa_np, wkva_np, wqb_f, wkvbn_np, wkvbv_np, wo_np,
            cosA_np, sinA2_np, swapM_np)


def run(inputs, trace=False, tmpdir=None, **kw):
    (xT_np, wqa_np, wkva_np, wqb_f, wkvbn_np, wkvbv_np, wo_np,
     cosA_np, sinA2_np, swapM_np) = _host_prep(**inputs)

    nc1 = _build_l1()
    in1 = []
    for i in range(N_CORES):
        sl = slice(i * TSL, (i + 1) * TSL)
        in1.append(dict(
            xTs=np.ascontiguousarray(xT_np[:, sl]),
            wqa=wqa_np, wkva=wkva_np, wqb=wqb_f, wkvbn=wkvbn_np,
            wkvbv=wkvbv_np,
            cosA=np.ascontiguousarray(cosA_np[:, sl]),
            sinA2=np.ascontiguousarray(sinA2_np[:, sl]),
            swapM=swapM_np,
        ))
    import os
    kw1 = dict(kw)
    if tmpdir:
        kw1["tmpdir"] = os.path.join(tmpdir, "l1")
        os.makedirs(kw1["tmpdir"], exist_ok=True)
    r1 = run_bass_kernel_spmd(nc1, in1, list(range(N_CORES)), trace=trace, **kw1)

    # host gather: ks/v over all tokens
    ks_full = np.concatenate([r1.results[i]["ksT_o"] for i in range(N_CORES)],
                             axis=2)                      # [H, 128, T]
    v_full = np.concatenate(
        [r1.results[i]["v_o"] for i in range(N_CORES)], axis=1)  # [H, 16, 128, VD]

    nc2 = _build_l2()
    in2 = []
    for i in range(N_CORES):
        in2.append(dict(
            qsT_i=np.ascontiguousarray(r1.results[i]["qsT_o"]),
            ksT_i=ks_full, v_i=v_full, wo_f=wo_np,
        ))
    kw2 = dict(kw)
    if tmpdir:
        kw2["tmpdir"] = os.path.join(tmpdir, "l2")
        os.makedirs(kw2["tmpdir"], exist_ok=True)
    r2 = run_bass_kernel_spmd(nc2, in2, list(range(N_CORES)), trace=trace, **kw2)

    out = np.concatenate([np.asarray(r2.results[i]["o"], np.float32)
                          for i in range(N_CORES)], axis=0)
    return out.reshape(1, T, D), r1, r2


def kernel(**inputs):
    out, _, _ = run(inputs)
    return out


# revision 14
# speedup vs baseline: 1.0085x; 1.0085x over previous
"""DeepSeek MLA attention (B=1, T=2048, D=2048, H=16) on 8 trn2 NeuronCores.

Two-launch sequence-parallel sharding (faster than plain head-TP: it avoids
replicating the q_a/kv_a input projections on every core).

Launch 1 (token-sharded): each core computes q/k/v for ALL 16 heads for its
256-token slice (no replicated projections). Launch 2 (query-sharded): each
core runs attention for its 256 queries over ALL heads and ALL 2048 keys,
then o_proj with the full wo, producing complete output rows (no all-reduce).
The host reshuffles k/v between launches (concat along tokens).
"""

import sys

if "/opt/trn_rl_repo" not in sys.path:
    sys.path.insert(0, "/opt/trn_rl_repo")

from contextlib import ExitStack

import ml_dtypes
import numpy as np

import concourse.bass as bass
import concourse.tile as tile
from concourse import bacc, mybir
from concourse.bass import ts
from concourse.bass_utils import run_bass_kernel_spmd
from concourse.masks import make_identity

BF16 = ml_dtypes.bfloat16
F32 = mybir.dt.float32
BF = mybir.dt.bfloat16

T, D = 2048, 2048
H, DK = 16, 128
ROPE, NOPE, VD = 64, 64, 128
HALF = ROPE // 2
KVR, QR = 512, 768
KVW = KVR + 128
SCALE = DK ** -0.5
EPS = float(np.finfo(np.float32).eps)
N_CORES = 8
TSL = T // N_CORES        # 256 tokens per core
NTT = T // 128            # 16 token tiles total
P = 128

QRT = QR // P   # 6
CRT = KVR // P  # 4
DT = D // P     # 16


def _emit_l1(nc, tc, ctx, aps):
    (xTs, wqa, wkva, wqb, wkvbn, wkvbv, cosA, sinA2, swapM,
     qsT_o, ksT_o, v_o) = aps
    TCH = TSL            # single 256-wide chunk
    TPC = TCH // P       # 2

    consts = ctx.enter_context(tc.tile_pool(name="consts", bufs=1))
    persist = ctx.enter_context(tc.tile_pool(name="persist", bufs=1))
    wpool = ctx.enter_context(tc.tile_pool(name="w1", bufs=1))
    rawpool = ctx.enter_context(tc.tile_pool(name="raw", bufs=1))
    sqpool = ctx.enter_context(tc.tile_pool(name="sq", bufs=3))
    scpool = ctx.enter_context(tc.tile_pool(name="scales", bufs=1))
    tmpool = ctx.enter_context(tc.tile_pool(name="ropetmp", bufs=2))
    outp = ctx.enter_context(tc.tile_pool(name="outs", bufs=4))
    ps_proj = ctx.enter_context(tc.tile_pool(name="ps_proj", bufs=3, space="PSUM"))
    ps_sum = ctx.enter_context(tc.tile_pool(name="ps_sum", bufs=1, space="PSUM"))
    ps_small = ctx.enter_context(tc.tile_pool(name="ps_small", bufs=3, space="PSUM"))

    ones128 = consts.tile([P, P], BF)
    nc.gpsimd.memset(ones128, 1.0)
    ident = consts.tile([P, P], F32)
    make_identity(nc, ident)
    eps_ap = consts.tile([P, 1], F32)
    nc.vector.memset(eps_ap, EPS)

    s_cT = persist.tile([P, TPC], F32)

    # interleave x tiles with projection weights so matmuls start early
    xc = persist.tile([P, DT, TCH], BF)
    wqa_sb = wpool.tile([P, DT, QR], BF)
    wkva_sb = wpool.tile([P, DT, KVW], BF)
    for dt in range(DT):
        nc.sync.dma_start(out=xc[:, dt, :], in_=xTs[ts(dt, P), :])
        nc.sync.dma_start(out=wqa_sb[:, dt, :], in_=wqa[ts(dt, P), :])
        nc.sync.dma_start(out=wkva_sb[:, dt, :], in_=wkva[ts(dt, P), :])
    wqb_sb = wpool.tile([P, QRT, H, DK], BF)
    nc.sync.dma_start(out=wqb_sb, in_=wqb.rearrange("(r p) h d -> p r h d", p=P))
    wkvbn_sb = wpool.tile([P, CRT, H, NOPE], BF)
    nc.sync.dma_start(out=wkvbn_sb, in_=wkvbn.rearrange("(r p) h d -> p r h d", p=P))
    wkvbv_sb = wpool.tile([P, CRT, H, VD], BF)
    nc.sync.dma_start(out=wkvbv_sb, in_=wkvbv.rearrange("(r p) h d -> p r h d", p=P))
    cosA_sb = wpool.tile([P, TCH], BF)
    nc.sync.dma_start(out=cosA_sb, in_=cosA)
    sinA2_sb = wpool.tile([P, TCH], BF)
    nc.sync.dma_start(out=sinA2_sb, in_=sinA2)
    swapM_sb = wpool.tile([P, P], BF)
    nc.sync.dma_start(out=swapM_sb, in_=swapM)

    qa_r = rawpool.tile([P, QRT, TCH], BF, tag="qa_raw")
    c_r = rawpool.tile([P, CRT, TCH], BF, tag="c_raw")
    kpe_r = rawpool.tile([P, TCH], BF, tag="kpe_raw")
    sq_q_ps = ps_sum.tile([P, TCH], F32, tag="sq_q")
    sq_c_ps = ps_sum.tile([P, TCH], F32, tag="sq_c")

    scale_q = scpool.tile([P, TCH], F32, tag="scale_q")
    scale_c = scpool.tile([P, TCH], F32, tag="scale_c")

    pending = None

    def flush_pending():
        nonlocal pending
        if pending is not None:
            tgt, sqt, st, sp = pending
            nc.tensor.matmul(tgt, ones128, sqt, start=st, stop=sp)
            pending = None

    for r in range(QRT + CRT + 1):
        ps = ps_proj.tile([P, TCH], F32, tag="proj")
        if r < QRT:
            w, col = wqa_sb, ts(r, P)
        elif r < QRT + CRT:
            w, col = wkva_sb, ts(r - QRT, P)
        else:
            w, col = wkva_sb, ts(CRT, P)
        for dt in range(DT):
            nc.tensor.matmul(ps, w[:, dt, col], xc[:, dt, :],
                             start=(dt == 0), stop=(dt == DT - 1))
        flush_pending()
        if r < QRT:
            nc.scalar.copy(out=qa_r[:, r, :], in_=ps)
            sq = sqpool.tile([P, TCH], BF, tag="sq")
            nc.scalar.activation(out=sq, in_=ps,
                                 func=mybir.ActivationFunctionType.Square)
            pending = (sq_q_ps, sq, r == 0, r == QRT - 1)
        elif r < QRT + CRT:
            rc = r - QRT
            nc.scalar.copy(out=c_r[:, rc, :], in_=ps)
            sq = sqpool.tile([P, TCH], BF, tag="sq")
            nc.scalar.activation(out=sq, in_=ps,
                                 func=mybir.ActivationFunctionType.Square)
            pending = (sq_c_ps, sq, rc == 0, rc == CRT - 1)
        else:
            nc.scalar.copy(out=kpe_r, in_=ps)
        if r == QRT:
            tmp_q = scpool.tile([P, TCH], F32, tag="scale_tmp")
            nc.scalar.activation(out=tmp_q, in_=sq_q_ps,
                                 func=mybir.ActivationFunctionType.Sqrt,
                                 scale=1.0 / QR, bias=eps_ap)
            nc.vector.reciprocal(out=scale_q, in_=tmp_q)
    flush_pending()
    tmp_c = scpool.tile([P, TCH], F32, tag="scale_tmp")
    nc.scalar.activation(out=tmp_c, in_=sq_c_ps,
                         func=mybir.ActivationFunctionType.Sqrt,
                         scale=1.0 / KVR, bias=eps_ap)
    nc.vector.reciprocal(out=scale_c, in_=tmp_c)

    cos_s = scpool.tile([P, TCH], BF, tag="cos_s")
    sin_s = scpool.tile([P, TCH], BF, tag="sin_s")
    nc.vector.tensor_mul(cos_s[64:128, :], cosA_sb[64:128, :], scale_q[64:128, :])
    nc.vector.tensor_mul(sin_s[64:128, :], sinA2_sb[64:128, :], scale_q[64:128, :])

    # --- all qsT matmul groups (PE-dense), evictions on ACT/DVE ---
    qs_all = persist.tile([P, H, TCH], BF)
    qpe_all = persist.tile([P, H, TCH], BF)
    for h in range(H):
        ps = ps_proj.tile([P, TCH], F32, tag="proj")
        for r in range(QRT):
            nc.tensor.matmul(ps, wqb_sb[:, r, h, :], qa_r[:, r, :],
                             start=(r == 0), stop=(r == QRT - 1))
        nc.vector.tensor_mul(qs_all[0:64, h, :], ps[0:64, :], scale_q[0:64, :])
        nc.scalar.copy(out=qpe_all[:, h, :], in_=ps)

    # k_nope groups
    ks_l = []
    for h in range(H):
        ks_sb = outp.tile([P, TCH], BF, tag=f"ks{h % 4}")
        kn_ps = ps_small.tile([64, TCH], F32, tag="small")
        for r in range(CRT):
            nc.tensor.matmul(kn_ps, wkvbn_sb[:, r, h, :], c_r[:, r, :],
                             start=(r == 0), stop=(r == CRT - 1))
        nc.vector.tensor_mul(ks_sb[0:64, :], kn_ps, scale_c[0:64, :])
        ks_l.append(ks_sb)

    # scale_c column layout for v scaling
    for j in range(TPC):
        tr_ps = ps_small.tile([P, P], F32, tag="small")
        nc.tensor.transpose(tr_ps, scale_c[:, ts(j, P)], ident)
        nc.vector.tensor_copy(out=s_cT[:, j:j + 1], in_=tr_ps[:, 0:1])

    # shared roped k_pe
    ksw_ps = ps_proj.tile([P, TCH], F32, tag="proj")
    nc.tensor.matmul(ksw_ps, swapM_sb, kpe_r, start=True, stop=True)
    kpe_rope = persist.tile([P, TCH], BF)
    km1 = tmpool.tile([P, TCH], BF, tag="rope_m1")
    km2 = tmpool.tile([P, TCH], BF, tag="rope_m2")
    nc.vector.tensor_mul(km1[64:128, :], kpe_r[64:128, :], cosA_sb[64:128, :])
    nc.vector.tensor_mul(km2[64:128, :], ksw_ps[64:128, :], sinA2_sb[64:128, :])
    nc.vector.tensor_add(kpe_rope[64:128, :], km1[64:128, :], km2[64:128, :])
    for h in range(H):
        nc.vector.tensor_copy(out=ks_l[h][64:128, :], in_=kpe_rope[64:128, :])
        nc.sync.dma_start(out=ksT_o[h], in_=ks_l[h])

    # q rope swaps last (inputs evicted long before)
    for h in range(H):
        sw_ps = ps_proj.tile([P, TCH], F32, tag="proj")
        nc.tensor.matmul(sw_ps, swapM_sb, qpe_all[:, h, :], start=True, stop=True)
        m1 = tmpool.tile([P, TCH], BF, tag="rope_m1")
        m2 = tmpool.tile([P, TCH], BF, tag="rope_m2")
        nc.vector.tensor_mul(m1[64:128, :], qpe_all[64:128, h, :], cos_s[64:128, :])
        nc.vector.tensor_mul(m2[64:128, :], sw_ps[64:128, :], sin_s[64:128, :])
        nc.vector.tensor_add(qs_all[64:128, h, :], m1[64:128, :], m2[64:128, :])
        nc.sync.dma_start(out=qsT_o[h], in_=qs_all[:, h, :])


    # v groups (scale folded into the ACT eviction)
    for h in range(H):
        for j in range(TPC):
            v_ps = ps_small.tile([P, VD], F32, tag="small")
            for r in range(CRT):
                nc.tensor.matmul(v_ps, c_r[:, r, ts(j, P)], wkvbv_sb[:, r, h, :],
                                 start=(r == 0), stop=(r == CRT - 1))
            v_sb = outp.tile([P, VD], BF, tag="v")
            nc.scalar.activation(out=v_sb, in_=v_ps,
                                 func=mybir.ActivationFunctionType.Copy,
                                 scale=s_cT[:, j:j + 1])
            nc.sync.dma_start(out=v_o[h, j], in_=v_sb)

def _emit_l2(nc, tc, ctx, aps):
    (qsT_i, ksT_i, v_i, wo_f, o_out) = aps
    KPACK = 4           # k-tiles packed per exp op (4*256 = 1024 cols, 2 banks)

    consts = ctx.enter_context(tc.tile_pool(name="consts", bufs=1))
    persist = ctx.enter_context(tc.tile_pool(name="persist", bufs=1))
    wpool = ctx.enter_context(tc.tile_pool(name="w2", bufs=1))

    ones128 = consts.tile([P, P], BF)
    nc.gpsimd.memset(ones128, 1.0)

    qs_all = persist.tile([P, H, TSL], BF)
    outT = persist.tile([P, H, TSL], BF)
    wo_sb = wpool.tile([P, H, D], BF)

    with tc.tile_pool(name="kv", bufs=4) as kvpool, \
         tc.tile_pool(name="probs", bufs=4) as ppool, \
         tc.tile_pool(name="recip", bufs=3) as recpool, \
         tc.tile_pool(name="ps_att", bufs=2, space="PSUM") as ps_att, \
         tc.tile_pool(name="ps_acc", bufs=2, space="PSUM") as ps_acc:
        pend = None    # (pT, base kt, av_ps, dn_ps, v_tile)

        def flush_av(nc, KPACK):
            nonlocal pend
            if pend is None:
                return
            pT, kb, av_p, dn_p, v_t = pend
            for kl in range(KPACK):
                kt = kb + kl
                nc.tensor.matmul(av_p, v_t[:, kt, :], pT[:, kl, :],
                                 start=(kt == 0), stop=(kt == NTT - 1))
                nc.tensor.matmul(dn_p, ones128, pT[:, kl, :],
                                 start=(kt == 0), stop=(kt == NTT - 1))
            pend = None

        finals = []
        for h in range(H):
            ks_h = kvpool.tile([P, T], BF, tag="ks")
            nc.sync.dma_start(out=ks_h, in_=ksT_i[h])
            v_h = kvpool.tile([P, NTT, VD], BF, tag="v")
            nc.sync.dma_start(out=v_h, in_=v_i[h].rearrange("kt p d -> p kt d"))
            if h == 0:
                # queries ride behind the first head's k/v
                nc.sync.dma_start(out=qs_all, in_=qsT_i.rearrange("h p t -> p h t"))
            # o_proj weights stream in per-head slices so they never
            # monopolize the DMA queue ahead of the next head's k/v
            nc.sync.dma_start(out=wo_sb[:, h, :], in_=wo_f[h])

            av_ps = ps_acc.tile([P, TSL], F32, tag="av")
            dn_ps = ps_acc.tile([P, TSL], F32, tag="dn")
            for kg in range(NTT // KPACK):
                sc_ps = ps_att.tile([P, KPACK, TSL], F32, tag="scores")
                for kl in range(KPACK):
                    kt = kg * KPACK + kl
                    nc.tensor.matmul(sc_ps[:, kl, :], ks_h[:, ts(kt, P)],
                                     qs_all[:, h, :], start=True, stop=True)
                pT = ppool.tile([P, KPACK, TSL], BF, tag="pT")
                nc.scalar.activation(out=pT, in_=sc_ps,
                                     func=mybir.ActivationFunctionType.Exp,
                                     scale=SCALE)
                flush_av(nc, KPACK)
                pend = (pT, kg * KPACK, av_ps, dn_ps, v_h)
                while finals:
                    f_av, f_dn, f_h = finals.pop()
                    rec = recpool.tile([P, TSL], F32, tag="rec")
                    nc.vector.reciprocal(out=rec, in_=f_dn)
                    nc.vector.tensor_mul(outT[:, f_h, :], f_av, rec)
            finals.append((av_ps, dn_ps, h))
        flush_av(nc, KPACK)
        for f_av, f_dn, f_h in finals:
            rec = recpool.tile([P, TSL], F32, tag="rec")
            nc.vector.reciprocal(out=rec, in_=f_dn)
            nc.vector.tensor_mul(outT[:, f_h, :], f_av, rec)

    with tc.tile_pool(name="osb", bufs=3) as opool, \
         tc.tile_pool(name="ps_o", bufs=2, space="PSUM") as ps_o:
        for qt in range(TSL // P):
            o_ps = ps_o.tile([P, D // 512, 512], F32, tag="o")
            for h in range(H):
                for dc in range(D // 512):
                    nc.tensor.matmul(o_ps[:, dc, :], outT[:, h, ts(qt, P)],
                                     wo_sb[:, h, ts(dc, 512)],
                                     start=(h == 0), stop=(h == H - 1))
            o_sb = opool.tile([P, D], F32, tag="osb")
            nc.vector.tensor_copy(out=o_sb, in_=o_ps)
            nc.sync.dma_start(out=o_out[ts(qt, P), :], in_=o_sb)


_CACHE = {}


def _build_l1():
    if "l1" in _CACHE:
        return _CACHE["l1"]
    nc = bacc.Bacc("TRN2", target_bir_lowering=False, debug=False,
                   num_devices=N_CORES)
    xTs = nc.dram_tensor("xTs", [D, TSL], BF, kind="ExternalInput").ap()
    wqa = nc.dram_tensor("wqa", [D, QR], BF, kind="ExternalInput").ap()
    wkva = nc.dram_tensor("wkva", [D, KVW], BF, kind="ExternalInput").ap()
    wqb = nc.dram_tensor("wqb", [QR, H, DK], BF, kind="ExternalInput").ap()
    wkvbn = nc.dram_tensor("wkvbn", [KVR, H, NOPE], BF, kind="ExternalInput").ap()
    wkvbv = nc.dram_tensor("wkvbv", [KVR, H, VD], BF, kind="ExternalInput").ap()
    cosA = nc.dram_tensor("cosA", [P, TSL], BF, kind="ExternalInput").ap()
    sinA2 = nc.dram_tensor("sinA2", [P, TSL], BF, kind="ExternalInput").ap()
    swapM = nc.dram_tensor("swapM", [P, P], BF, kind="ExternalInput").ap()
    qsT_o = nc.dram_tensor("qsT_o", [H, P, TSL], BF, kind="ExternalOutput").ap()
    ksT_o = nc.dram_tensor("ksT_o", [H, P, TSL], BF, kind="ExternalOutput").ap()
    v_o = nc.dram_tensor("v_o", [H, TSL // P, P, VD], BF, kind="ExternalOutput").ap()
    aps = (xTs, wqa, wkva, wqb, wkvbn, wkvbv, cosA, sinA2, swapM,
           qsT_o, ksT_o, v_o)
    with tile.TileContext(nc) as tc, ExitStack() as ctx:
        _emit_l1(nc, tc, ctx, aps)
    nc.compile()
    _CACHE["l1"] = nc
    return nc


def _build_l2():
    if "l2" in _CACHE:
        return _CACHE["l2"]
    nc = bacc.Bacc("TRN2", target_bir_lowering=False, debug=False,
                   num_devices=N_CORES)
    qsT_i = nc.dram_tensor("qsT_i", [H, P, TSL], BF, kind="ExternalInput").ap()
    ksT_i = nc.dram_tensor("ksT_i", [H, P, T], BF, kind="ExternalInput").ap()
    v_i = nc.dram_tensor("v_i", [H, NTT, P, VD], BF, kind="ExternalInput").ap()
    wo_f = nc.dram_tensor("wo_f", [H, VD, D], BF, kind="ExternalInput").ap()
    o_out = nc.dram_tensor("o", [TSL, D], F32, kind="ExternalOutput").ap()
    aps = (qsT_i, ksT_i, v_i, wo_f, o_out)
    with tile.TileContext(nc) as tc, ExitStack() as ctx:
        _emit_l2(nc, tc, ctx, aps)
    nc.compile()
    _CACHE["l2"] = nc
    return nc


def _host_prep(x, wq_a, q_a_norm_w, wq_b, wkv_a, kv_a_norm_w, wkv_b, wo):
    x2 = np.asarray(x, np.float32).reshape(T, D)
    xT_np = np.ascontiguousarray(x2.T).astype(BF16)
    wqa_np = np.asarray(wq_a, np.float32).astype(BF16)
    wkva_f = np.asarray(wkv_a, np.float32)
    wkva_np = np.zeros((D, KVW), BF16)
    wkva_np[:, :KVR] = wkva_f[:, :KVR].astype(BF16)
    wkva_np[:, KVR + 64:] = wkva_f[:, KVR:].astype(BF16)
    wqb_f = (np.asarray(q_a_norm_w, np.float32)[:, None]
             * np.asarray(wq_b, np.float32)).reshape(QR, H, DK).astype(BF16)
    wkvb_f = (np.asarray(kv_a_norm_w, np.float32)[:, None]
              * np.asarray(wkv_b, np.float32)).reshape(KVR, H, NOPE + VD)
    wkvbn_np = np.ascontiguousarray(wkvb_f[:, :, :NOPE]).astype(BF16)
    wkvbv_np = np.ascontiguousarray(wkvb_f[:, :, NOPE:]).astype(BF16)
    wo_np = np.asarray(wo, np.float32).reshape(H, VD, D).astype(BF16)

    inv_freq = 1.0 / (10000.0 ** (np.arange(0, ROPE, 2, dtype=np.float32) / ROPE))
    tpos = np.arange(T, dtype=np.float32)
    freqs = np.outer(inv_freq, tpos)
    cos = np.cos(freqs).astype(np.float32)
    sin = np.sin(freqs).astype(np.float32)
    cosA_np = np.concatenate([cos, cos, cos, cos], 0).astype(BF16)
    sin2 = np.concatenate([-sin, sin], 0)
    sinA2_np = np.concatenate([sin2, sin2], 0).astype(BF16)
    swap_np = np.zeros((P, P), np.float32)
    for i in range(HALF):
        swap_np[64 + HALF + i, 64 + i] = 1.0
        swap_np[64 + i, 64 + HALF + i] = 1.0
    swapM_np = swap_np.astype(BF16)
    return (xT_np, wqa_np, wkva_np, wqb_f, wkvbn_np, wkvbv_np, wo_np,
            cosA_np, sinA2_np, swapM_np)


def run(inputs, trace=False, tmpdir=None, **kw):
    (xT_np, wqa_np, wkva_np, wqb_f, wkvbn_np, wkvbv_np, wo_np,
     cosA_np, sinA2_np, swapM_np) = _host_prep(**inputs)

    nc1 = _build_l1()
    in1 = []
    for i in range(N_CORES):
        sl = slice(i * TSL, (i + 1) * TSL)
        in1.append(dict(
            xTs=np.ascontiguousarray(xT_np[:, sl]),
            wqa=wqa_np, wkva=wkva_np, wqb=wqb_f, wkvbn=wkvbn_np,
            wkvbv=wkvbv_np,
            cosA=np.ascontiguousarray(cosA_np[:, sl]),
            sinA2=np.ascontiguousarray(sinA2_np[:, sl]),
            swapM=swapM_np,
        ))
    import os
    kw1 = dict(kw)
    if tmpdir:
        kw1["tmpdir"] = os.path.join(tmpdir, "l1")
        os.makedirs(kw1["tmpdir"], exist_ok=True)
    r1 = run_bass_kernel_spmd(nc1, in1, list(range(N_CORES)), trace=trace, **kw1)

    # host gather: ks/v over all tokens
    ks_full = np.concatenate([r1.results[i]["ksT_o"] for i in range(N_CORES)],
                             axis=2)                      # [H, 128, T]
    v_full = np.concatenate(
        [r1.results[i]["v_o"] for i in range(N_CORES)], axis=1)  # [H, 16, 128, VD]

    nc2 = _build_l2()
    in2 = []
    for i in range(N_CORES):
        in2.append(dict(
            qsT_i=np.ascontiguousarray(r1.results[i]["qsT_o"]),
            ksT_i=ks_full, v_i=v_full, wo_f=wo_np,
        ))
    kw2 = dict(kw)
    if tmpdir:
        kw2["tmpdir"] = os.path.join(tmpdir, "l2")
        os.makedirs(kw2["tmpdir"], exist_ok=True)
    r2 = run_bass_kernel_spmd(nc2, in2, list(range(N_CORES)), trace=trace, **kw2)

    out = np.concatenate([np.asarray(r2.results[i]["o"], np.float32)
                          for i in range(N_CORES)], axis=0)
    return out.reshape(1, T, D), r1, r2


def kernel(**inputs):
    out, _, _ = run(inputs)
    return out


# revision 16
# speedup vs baseline: 1.1076x; 1.0983x over previous
"""DeepSeek MLA attention (B=1, T=2048, D=2048, H=16) on 8 trn2 NeuronCores.

Two-launch sequence-parallel sharding (faster than plain head-TP: it avoids
replicating the q_a/kv_a input projections on every core).

Launch 1 (token-sharded): each core computes q/k/v for ALL 16 heads for its
256-token slice (no replicated projections). Launch 2 (query-sharded): each
core runs attention for its 256 queries over ALL heads and ALL 2048 keys,
then o_proj with the full wo, producing complete output rows (no all-reduce).
The host reshuffles k/v between launches (concat along tokens).
"""

import sys

if "/opt/trn_rl_repo" not in sys.path:
    sys.path.insert(0, "/opt/trn_rl_repo")

from contextlib import ExitStack

import ml_dtypes
import numpy as np

import concourse.bass as bass
import concourse.tile as tile
from concourse import bacc, mybir
from concourse.bass import ts
from concourse.bass_utils import run_bass_kernel_spmd
from concourse.masks import make_identity

BF16 = ml_dtypes.bfloat16
F32 = mybir.dt.float32
BF = mybir.dt.bfloat16

T, D = 2048, 2048
H, DK = 16, 128
ROPE, NOPE, VD = 64, 64, 128
HALF = ROPE // 2
KVR, QR = 512, 768
KVW = KVR + 128
SCALE = DK ** -0.5
EPS = float(np.finfo(np.float32).eps)
N_CORES = 8
TSL = T // N_CORES        # 256 tokens per core
NTT = T // 128            # 16 token tiles total
P = 128

QRT = QR // P   # 6
CRT = KVR // P  # 4
DT = D // P     # 16


def _emit_l1(nc, tc, ctx, aps):
    (xTs, wqa, wkva, wqb, wkvbn, wkvbv, cosA, sinA2, swapM,
     qsT_o, ksT_o, v_o) = aps
    TCH = TSL            # single 256-wide chunk
    TPC = TCH // P       # 2

    consts = ctx.enter_context(tc.tile_pool(name="consts", bufs=1))
    persist = ctx.enter_context(tc.tile_pool(name="persist", bufs=1))
    wpool = ctx.enter_context(tc.tile_pool(name="w1", bufs=1))
    rawpool = ctx.enter_context(tc.tile_pool(name="raw", bufs=1))
    sqpool = ctx.enter_context(tc.tile_pool(name="sq", bufs=3))
    scpool = ctx.enter_context(tc.tile_pool(name="scales", bufs=1))
    tmpool = ctx.enter_context(tc.tile_pool(name="ropetmp", bufs=2))
    outp = ctx.enter_context(tc.tile_pool(name="outs", bufs=4))
    ps_proj = ctx.enter_context(tc.tile_pool(name="ps_proj", bufs=3, space="PSUM"))
    ps_sum = ctx.enter_context(tc.tile_pool(name="ps_sum", bufs=1, space="PSUM"))
    ps_small = ctx.enter_context(tc.tile_pool(name="ps_small", bufs=3, space="PSUM"))

    ones128 = consts.tile([P, P], BF)
    nc.gpsimd.memset(ones128, 1.0)
    ident = consts.tile([P, P], F32)
    make_identity(nc, ident)
    eps_ap = consts.tile([P, 1], F32)
    nc.vector.memset(eps_ap, EPS)

    s_cT = persist.tile([P, TPC], F32)

    # interleave x tiles with projection weights so matmuls start early
    xc = persist.tile([P, DT, TCH], BF)
    wqa_sb = wpool.tile([P, DT, QR], BF)
    wkva_sb = wpool.tile([P, DT, KVW], BF)
    for dt in range(DT):
        nc.sync.dma_start(out=xc[:, dt, :], in_=xTs[ts(dt, P), :])
        nc.sync.dma_start(out=wqa_sb[:, dt, :], in_=wqa[ts(dt, P), :])
        nc.sync.dma_start(out=wkva_sb[:, dt, :], in_=wkva[ts(dt, P), :])
    wqb_sb = wpool.tile([P, QRT, H, DK], BF)
    nc.sync.dma_start(out=wqb_sb, in_=wqb.rearrange("(r p) h d -> p r h d", p=P))
    wkvbn_sb = wpool.tile([P, CRT, H, NOPE], BF)
    nc.sync.dma_start(out=wkvbn_sb, in_=wkvbn.rearrange("(r p) h d -> p r h d", p=P))
    wkvbv_sb = wpool.tile([P, CRT, H, VD], BF)
    nc.sync.dma_start(out=wkvbv_sb, in_=wkvbv.rearrange("(r p) h d -> p r h d", p=P))
    cosA_sb = wpool.tile([P, TCH], BF)
    nc.sync.dma_start(out=cosA_sb, in_=cosA)
    sinA2_sb = wpool.tile([P, TCH], BF)
    nc.sync.dma_start(out=sinA2_sb, in_=sinA2)
    swapM_sb = wpool.tile([P, P], BF)
    nc.sync.dma_start(out=swapM_sb, in_=swapM)

    qa_r = rawpool.tile([P, QRT, TCH], BF, tag="qa_raw")
    c_r = rawpool.tile([P, CRT, TCH], BF, tag="c_raw")
    kpe_r = rawpool.tile([P, TCH], BF, tag="kpe_raw")
    sq_q_ps = ps_sum.tile([P, TCH], F32, tag="sq_q")
    sq_c_ps = ps_sum.tile([P, TCH], F32, tag="sq_c")

    scale_q = scpool.tile([P, TCH], F32, tag="scale_q")
    scale_c = scpool.tile([P, TCH], F32, tag="scale_c")

    pending = None

    def flush_pending():
        nonlocal pending
        if pending is not None:
            tgt, sqt, st, sp = pending
            nc.tensor.matmul(tgt, ones128, sqt, start=st, stop=sp)
            pending = None

    for r in range(QRT + CRT + 1):
        ps = ps_proj.tile([P, TCH], F32, tag="proj")
        if r < QRT:
            w, col = wqa_sb, ts(r, P)
        elif r < QRT + CRT:
            w, col = wkva_sb, ts(r - QRT, P)
        else:
            w, col = wkva_sb, ts(CRT, P)
        for dt in range(DT):
            nc.tensor.matmul(ps, w[:, dt, col], xc[:, dt, :],
                             start=(dt == 0), stop=(dt == DT - 1))
        flush_pending()
        if r < QRT:
            nc.scalar.copy(out=qa_r[:, r, :], in_=ps)
            sq = sqpool.tile([P, TCH], BF, tag="sq")
            nc.scalar.activation(out=sq, in_=ps,
                                 func=mybir.ActivationFunctionType.Square)
            pending = (sq_q_ps, sq, r == 0, r == QRT - 1)
        elif r < QRT + CRT:
            rc = r - QRT
            nc.scalar.copy(out=c_r[:, rc, :], in_=ps)
            sq = sqpool.tile([P, TCH], BF, tag="sq")
            nc.scalar.activation(out=sq, in_=ps,
                                 func=mybir.ActivationFunctionType.Square)
            pending = (sq_c_ps, sq, rc == 0, rc == CRT - 1)
        else:
            nc.scalar.copy(out=kpe_r, in_=ps)
        if r == QRT:
            tmp_q = scpool.tile([P, TCH], F32, tag="scale_tmp")
            nc.scalar.activation(out=tmp_q, in_=sq_q_ps,
                                 func=mybir.ActivationFunctionType.Sqrt,
                                 scale=1.0 / QR, bias=eps_ap)
            nc.vector.reciprocal(out=scale_q, in_=tmp_q)
    flush_pending()
    tmp_c = scpool.tile([P, TCH], F32, tag="scale_tmp")
    nc.scalar.activation(out=tmp_c, in_=sq_c_ps,
                         func=mybir.ActivationFunctionType.Sqrt,
                         scale=1.0 / KVR, bias=eps_ap)
    nc.vector.reciprocal(out=scale_c, in_=tmp_c)

    cos_s = scpool.tile([P, TCH], BF, tag="cos_s")
    sin_s = scpool.tile([P, TCH], BF, tag="sin_s")
    nc.vector.tensor_mul(cos_s[64:128, :], cosA_sb[64:128, :], scale_q[64:128, :])
    nc.vector.tensor_mul(sin_s[64:128, :], sinA2_sb[64:128, :], scale_q[64:128, :])

    # --- all qsT matmul groups (PE-dense), evictions on ACT/DVE ---
    qs_all = persist.tile([P, H, TCH], BF)
    qpe_all = persist.tile([P, H, TCH], BF)
    for h in range(H):
        ps = ps_proj.tile([P, TCH], F32, tag="proj")
        for r in range(QRT):
            nc.tensor.matmul(ps, wqb_sb[:, r, h, :], qa_r[:, r, :],
                             start=(r == 0), stop=(r == QRT - 1))
        nc.vector.tensor_mul(qs_all[0:64, h, :], ps[0:64, :], scale_q[0:64, :])
        nc.scalar.copy(out=qpe_all[:, h, :], in_=ps)

    # k_nope groups
    ks_all = persist.tile([P, H, TCH], BF)
    for h in range(H):
        kn_ps = ps_small.tile([64, TCH], F32, tag="small")
        for r in range(CRT):
            nc.tensor.matmul(kn_ps, wkvbn_sb[:, r, h, :], c_r[:, r, :],
                             start=(r == 0), stop=(r == CRT - 1))
        nc.vector.tensor_mul(ks_all[0:64, h, :], kn_ps, scale_c[0:64, :])

    # scale_c column layout for v scaling
    for j in range(TPC):
        tr_ps = ps_small.tile([P, P], F32, tag="small")
        nc.tensor.transpose(tr_ps, scale_c[:, ts(j, P)], ident)
        nc.vector.tensor_copy(out=s_cT[:, j:j + 1], in_=tr_ps[:, 0:1])

    # shared roped k_pe
    ksw_ps = ps_proj.tile([P, TCH], F32, tag="proj")
    nc.tensor.matmul(ksw_ps, swapM_sb, kpe_r, start=True, stop=True)
    kpe_rope = persist.tile([P, TCH], BF)
    km1 = tmpool.tile([P, TCH], BF, tag="rope_m1")
    km2 = tmpool.tile([P, TCH], BF, tag="rope_m2")
    nc.vector.tensor_mul(km1[64:128, :], kpe_r[64:128, :], cosA_sb[64:128, :])
    nc.vector.tensor_mul(km2[64:128, :], ksw_ps[64:128, :], sinA2_sb[64:128, :])
    nc.vector.tensor_add(kpe_rope[64:128, :], km1[64:128, :], km2[64:128, :])
    for h in range(H):
        nc.vector.tensor_copy(out=ks_all[64:128, h, :],
                              in_=kpe_rope[64:128, :])
    for g in range(2):
        nc.sync.dma_start(
            out=ksT_o[g * 8:(g + 1) * 8].rearrange("h p t -> p h t"),
            in_=ks_all[:, g * 8:(g + 1) * 8, :],
        )

    # q rope swaps last (inputs evicted long before)
    for h in range(H):
        sw_ps = ps_proj.tile([P, TCH], F32, tag="proj")
        nc.tensor.matmul(sw_ps, swapM_sb, qpe_all[:, h, :], start=True, stop=True)
        m1 = tmpool.tile([P, TCH], BF, tag="rope_m1")
        m2 = tmpool.tile([P, TCH], BF, tag="rope_m2")
        nc.vector.tensor_mul(m1[64:128, :], qpe_all[64:128, h, :], cos_s[64:128, :])
        nc.vector.tensor_mul(m2[64:128, :], sw_ps[64:128, :], sin_s[64:128, :])
        nc.vector.tensor_add(qs_all[64:128, h, :], m1[64:128, :], m2[64:128, :])
        if h % 8 == 7:
            g = h // 8
            nc.sync.dma_start(
                out=qsT_o[g * 8:(g + 1) * 8].rearrange("h p t -> p h t"),
                in_=qs_all[:, g * 8:(g + 1) * 8, :],
            )


    # v groups (scale folded into the ACT eviction)
    v_all = persist.tile([P, H, TPC, VD], BF)
    for h in range(H):
        for j in range(TPC):
            v_ps = ps_small.tile([P, VD], F32, tag="small")
            for r in range(CRT):
                nc.tensor.matmul(v_ps, c_r[:, r, ts(j, P)], wkvbv_sb[:, r, h, :],
                                 start=(r == 0), stop=(r == CRT - 1))
            nc.scalar.activation(out=v_all[:, h, j, :], in_=v_ps,
                                 func=mybir.ActivationFunctionType.Copy,
                                 scale=s_cT[:, j:j + 1])
        if h % 8 == 7:
            g = h // 8
            nc.sync.dma_start(
                out=v_o[g * 8:(g + 1) * 8].rearrange("h j p d -> p h j d"),
                in_=v_all[:, g * 8:(g + 1) * 8, :, :],
            )

def _emit_l2(nc, tc, ctx, aps):
    (qsT_i, ksT_i, v_i, wo_f, o_out) = aps
    KPACK = 4           # k-tiles packed per exp op (4*256 = 1024 cols, 2 banks)

    consts = ctx.enter_context(tc.tile_pool(name="consts", bufs=1))
    persist = ctx.enter_context(tc.tile_pool(name="persist", bufs=1))
    wpool = ctx.enter_context(tc.tile_pool(name="w2", bufs=1))

    ones128 = consts.tile([P, P], BF)
    nc.gpsimd.memset(ones128, 1.0)

    qs_all = persist.tile([P, H, TSL], BF)
    outT = persist.tile([P, H, TSL], BF)
    wo_sb = wpool.tile([P, H, D], BF)

    with tc.tile_pool(name="kv", bufs=4) as kvpool, \
         tc.tile_pool(name="probs", bufs=4) as ppool, \
         tc.tile_pool(name="recip", bufs=3) as recpool, \
         tc.tile_pool(name="ps_att", bufs=2, space="PSUM") as ps_att, \
         tc.tile_pool(name="ps_acc", bufs=2, space="PSUM") as ps_acc:
        pend = None    # (pT, base kt, av_ps, dn_ps, v_tile)

        def flush_av(nc, KPACK):
            nonlocal pend
            if pend is None:
                return
            pT, kb, av_p, dn_p, v_t = pend
            for kl in range(KPACK):
                kt = kb + kl
                nc.tensor.matmul(av_p, v_t[:, kt, :], pT[:, kl, :],
                                 start=(kt == 0), stop=(kt == NTT - 1))
                nc.tensor.matmul(dn_p, ones128, pT[:, kl, :],
                                 start=(kt == 0), stop=(kt == NTT - 1))
            pend = None

        finals = []
        for h in range(H):
            ks_h = kvpool.tile([P, T], BF, tag="ks")
            nc.sync.dma_start(out=ks_h, in_=ksT_i[h])
            v_h = kvpool.tile([P, NTT, VD], BF, tag="v")
            nc.sync.dma_start(out=v_h, in_=v_i[h].rearrange("kt p d -> p kt d"))
            if h == 0:
                # queries ride behind the first head's k/v
                nc.sync.dma_start(out=qs_all, in_=qsT_i.rearrange("h p t -> p h t"))
            # o_proj weights stream in per-head slices so they never
            # monopolize the DMA queue ahead of the next head's k/v
            nc.sync.dma_start(out=wo_sb[:, h, :], in_=wo_f[h])

            av_ps = ps_acc.tile([P, TSL], F32, tag="av")
            dn_ps = ps_acc.tile([P, TSL], F32, tag="dn")
            for kg in range(NTT // KPACK):
                sc_ps = ps_att.tile([P, KPACK, TSL], F32, tag="scores")
                for kl in range(KPACK):
                    kt = kg * KPACK + kl
                    nc.tensor.matmul(sc_ps[:, kl, :], ks_h[:, ts(kt, P)],
                                     qs_all[:, h, :], start=True, stop=True)
                pT = ppool.tile([P, KPACK, TSL], BF, tag="pT")
                nc.scalar.activation(out=pT, in_=sc_ps,
                                     func=mybir.ActivationFunctionType.Exp,
                                     scale=SCALE)
                flush_av(nc, KPACK)
                pend = (pT, kg * KPACK, av_ps, dn_ps, v_h)
                while finals:
                    f_av, f_dn, f_h = finals.pop()
                    rec = recpool.tile([P, TSL], F32, tag="rec")
                    nc.vector.reciprocal(out=rec, in_=f_dn)
                    nc.vector.tensor_mul(outT[:, f_h, :], f_av, rec)
            finals.append((av_ps, dn_ps, h))
        flush_av(nc, KPACK)
        for f_av, f_dn, f_h in finals:
            rec = recpool.tile([P, TSL], F32, tag="rec")
            nc.vector.reciprocal(out=rec, in_=f_dn)
            nc.vector.tensor_mul(outT[:, f_h, :], f_av, rec)

    with tc.tile_pool(name="osb", bufs=3) as opool, \
         tc.tile_pool(name="ps_o", bufs=2, space="PSUM") as ps_o:
        for qt in range(TSL // P):
            o_ps = ps_o.tile([P, D // 512, 512], F32, tag="o")
            for h in range(H):
                for dc in range(D // 512):
                    nc.tensor.matmul(o_ps[:, dc, :], outT[:, h, ts(qt, P)],
                                     wo_sb[:, h, ts(dc, 512)],
                                     start=(h == 0), stop=(h == H - 1))
            o_sb = opool.tile([P, D], F32, tag="osb")
            nc.vector.tensor_copy(out=o_sb, in_=o_ps)
            nc.sync.dma_start(out=o_out[ts(qt, P), :], in_=o_sb)


_CACHE = {}


def _build_l1():
    if "l1" in _CACHE:
        return _CACHE["l1"]
    nc = bacc.Bacc("TRN2", target_bir_lowering=False, debug=False,
                   num_devices=N_CORES)
    xTs = nc.dram_tensor("xTs", [D, TSL], BF, kind="ExternalInput").ap()
    wqa = nc.dram_tensor("wqa", [D, QR], BF, kind="ExternalInput").ap()
    wkva = nc.dram_tensor("wkva", [D, KVW], BF, kind="ExternalInput").ap()
    wqb = nc.dram_tensor("wqb", [QR, H, DK], BF, kind="ExternalInput").ap()
    wkvbn = nc.dram_tensor("wkvbn", [KVR, H, NOPE], BF, kind="ExternalInput").ap()
    wkvbv = nc.dram_tensor("wkvbv", [KVR, H, VD], BF, kind="ExternalInput").ap()
    cosA = nc.dram_tensor("cosA", [P, TSL], BF, kind="ExternalInput").ap()
    sinA2 = nc.dram_tensor("sinA2", [P, TSL], BF, kind="ExternalInput").ap()
    swapM = nc.dram_tensor("swapM", [P, P], BF, kind="ExternalInput").ap()
    qsT_o = nc.dram_tensor("qsT_o", [H, P, TSL], BF, kind="ExternalOutput").ap()
    ksT_o = nc.dram_tensor("ksT_o", [H, P, TSL], BF, kind="ExternalOutput").ap()
    v_o = nc.dram_tensor("v_o", [H, TSL // P, P, VD], BF, kind="ExternalOutput").ap()
    aps = (xTs, wqa, wkva, wqb, wkvbn, wkvbv, cosA, sinA2, swapM,
           qsT_o, ksT_o, v_o)
    with tile.TileContext(nc) as tc, ExitStack() as ctx:
        _emit_l1(nc, tc, ctx, aps)
    nc.compile()
    _CACHE["l1"] = nc
    return nc


def _build_l2():
    if "l2" in _CACHE:
        return _CACHE["l2"]
    nc = bacc.Bacc("TRN2", target_bir_lowering=False, debug=False,
                   num_devices=N_CORES)
    qsT_i = nc.dram_tensor("qsT_i", [H, P, TSL], BF, kind="ExternalInput").ap()
    ksT_i = nc.dram_tensor("ksT_i", [H, P, T], BF, kind="ExternalInput").ap()
    v_i = nc.dram_tensor("v_i", [H, NTT, P, VD], BF, kind="ExternalInput").ap()
    wo_f = nc.dram_tensor("wo_f", [H, VD, D], BF, kind="ExternalInput").ap()
    o_out = nc.dram_tensor("o", [TSL, D], F32, kind="ExternalOutput").ap()
    aps = (qsT_i, ksT_i, v_i, wo_f, o_out)
    with tile.TileContext(nc) as tc, ExitStack() as ctx:
        _emit_l2(nc, tc, ctx, aps)
    nc.compile()
    _CACHE["l2"] = nc
    return nc


def _host_prep(x, wq_a, q_a_norm_w, wq_b, wkv_a, kv_a_norm_w, wkv_b, wo):
    x2 = np.asarray(x, np.float32).reshape(T, D)
    xT_np = np.ascontiguousarray(x2.T).astype(BF16)
    wqa_np = np.asarray(wq_a, np.float32).astype(BF16)
    wkva_f = np.asarray(wkv_a, np.float32)
    wkva_np = np.zeros((D, KVW), BF16)
    wkva_np[:, :KVR] = wkva_f[:, :KVR].astype(BF16)
    wkva_np[:, KVR + 64:] = wkva_f[:, KVR:].astype(BF16)
    wqb_f = (np.asarray(q_a_norm_w, np.float32)[:, None]
             * np.asarray(wq_b, np.float32)).reshape(QR, H, DK).astype(BF16)
    wkvb_f = (np.asarray(kv_a_norm_w, np.float32)[:, None]
              * np.asarray(wkv_b, np.float32)).reshape(KVR, H, NOPE + VD)
    wkvbn_np = np.ascontiguousarray(wkvb_f[:, :, :NOPE]).astype(BF16)
    wkvbv_np = np.ascontiguousarray(wkvb_f[:, :, NOPE:]).astype(BF16)
    wo_np = np.asarray(wo, np.float32).reshape(H, VD, D).astype(BF16)

    inv_freq = 1.0 / (10000.0 ** (np.arange(0, ROPE, 2, dtype=np.float32) / ROPE))
    tpos = np.arange(T, dtype=np.float32)
    freqs = np.outer(inv_freq, tpos)
    cos = np.cos(freqs).astype(np.float32)
    sin = np.sin(freqs).astype(np.float32)
    cosA_np = np.concatenate([cos, cos, cos, cos], 0).astype(BF16)
    sin2 = np.concatenate([-sin, sin], 0)
    sinA2_np = np.concatenate([sin2, sin2], 0).astype(BF16)
    swap_np = np.zeros((P, P), np.float32)
    for i in range(HALF):
        swap_np[64 + HALF + i, 64 + i] = 1.0
        swap_np[64 + i, 64 + HALF + i] = 1.0
    swapM_np = swap_np.astype(BF16)
    return (xT_np, wqa_np, wkva_np, wqb_f, wkvbn_np, wkvbv_np, wo_np,
            cosA_np, sinA2_np, swapM_np)


def run(inputs, trace=False, tmpdir=None, **kw):
    (xT_np, wqa_np, wkva_np, wqb_f, wkvbn_np, wkvbv_np, wo_np,
     cosA_np, sinA2_np, swapM_np) = _host_prep(**inputs)

    nc1 = _build_l1()
    in1 = []
    for i in range(N_CORES):
        sl = slice(i * TSL, (i + 1) * TSL)
        in1.append(dict(
            xTs=np.ascontiguousarray(xT_np[:, sl]),
            wqa=wqa_np, wkva=wkva_np, wqb=wqb_f, wkvbn=wkvbn_np,
            wkvbv=wkvbv_np,
            cosA=np.ascontiguousarray(cosA_np[:, sl]),
            sinA2=np.ascontiguousarray(sinA2_np[:, sl]),
            swapM=swapM_np,
        ))
    import os
    kw1 = dict(kw)
    if tmpdir:
        kw1["tmpdir"] = os.path.join(tmpdir, "l1")
        os.makedirs(kw1["tmpdir"], exist_ok=True)
    r1 = run_bass_kernel_spmd(nc1, in1, list(range(N_CORES)), trace=trace, **kw1)

    # host gather: ks/v over all tokens
    ks_full = np.concatenate([r1.results[i]["ksT_o"] for i in range(N_CORES)],
                             axis=2)                      # [H, 128, T]
    v_full = np.concatenate(
        [r1.results[i]["v_o"] for i in range(N_CORES)], axis=1)  # [H, 16, 128, VD]

    nc2 = _build_l2()
    in2 = []
    for i in range(N_CORES):
        in2.append(dict(
            qsT_i=np.ascontiguousarray(r1.results[i]["qsT_o"]),
            ksT_i=ks_full, v_i=v_full, wo_f=wo_np,
        ))
    kw2 = dict(kw)
    if tmpdir:
        kw2["tmpdir"] = os.path.join(tmpdir, "l2")
        os.makedirs(kw2["tmpdir"], exist_ok=True)
    r2 = run_bass_kernel_spmd(nc2, in2, list(range(N_CORES)), trace=trace, **kw2)

    out = np.concatenate([np.asarray(r2.results[i]["o"], np.float32)
                          for i in range(N_CORES)], axis=0)
    return out.reshape(1, T, D), r1, r2


def kernel(**inputs):
    out, _, _ = run(inputs)
    return out
